# revision 3
# baseline (speedup 1.0000x reference)
"""Trainium2 Bass kernel for nn_Decoder (Tacotron-style decoder, 200-step scan).

Strategy: data-parallel over batch (B=16 -> 8 cores x 2 sequences each); all
weights resident in SBUF; one For_i hardware loop over the 200 decoder steps.
The LSTM state resets every step, so the only recurrent state is the
cumulative attention weights (awc) and h2. The forget gate is dead
(c_prev == 0) and is dropped. sigmoid(x) = 0.5*tanh(x/2)+0.5 so tanh+exp (one
ACT table set) cover all nonlinearities; scale factors are folded into
weights on the host (i/o gate rows x0.5; h' = 2h absorbed into consumers).

Host precomputes: prenet + x-part of LSTM0 gates for all steps, proc_enc,
the fused location conv+dense kernel W2 = loc_dense_w @ loc_conv_w, masks.
"""
import sys
sys.path.insert(0, "/opt/trn_rl_repo")
import numpy as np

import concourse.bass as bass
import concourse.tile as tile
from concourse import bacc, mybir
from concourse.bass_utils import run_bass_kernel_spmd

B, T_ENC, T_DEC = 16, 256, 200
E, ATT, PRE, H, MEL, LOC, K = 512, 128, 256, 1024, 80, 32, 31
NCORES = 8
BPC = B // NCORES          # 2 sequences per core
G3 = 3 * H                 # 3072 gates per layer (i, o, g)
NSLOT = G3 // 128          # 24
PAD = (K - 1) // 2         # 15
TPADDED = T_ENC + 2 * PAD  # 286
NEG = -30.0                # mask offset inside exp

F32 = mybir.dt.float32
_cache = {}


def _build_program():
    nc = bacc.Bacc("TRN2", target_bir_lowering=False, debug=False,
                   enable_asserts=False, num_devices=NCORES)
    dt = F32

    def din(name, shape):
        return nc.dram_tensor(name, list(shape), dt, kind="ExternalInput").ap()

    # all inputs pre-laid-out on host to match SBUF [partition, free] exactly
    lpT = din("lpT", [128, 8 * ATT])
    w2T = din("w2T", [K, ATT])
    ewT = din("ewT", [ATT, 1])
    negc = din("negc", [1, 1])
    onesrow = din("onesrow", [1, 128])
    unmask = din("unmask", [1, BPC * T_ENC])
    maskT = din("maskT", [128, 2 * BPC])
    peT = din("peT", [ATT, BPC * T_ENC])
    encT = din("encT", [128, BPC * 2 * 4 * 128])
    w0cT = din("w0cT", [128, 4 * NSLOT * 128])
    w1T = din("w1T", [128, 8 * NSLOT * 128])
    decwT = din("decwT", [128, 12 * (MEL + 1)])
    x0g = din("x0g", [T_DEC, 128, 2 * NSLOT])
    out_ext = nc.dram_tensor("out", [MEL + 1, T_DEC * BPC], dt,
                             kind="ExternalOutput").ap()

    Tanh = mybir.ActivationFunctionType.Tanh
    Exp = mybir.ActivationFunctionType.Exp
    MUL = mybir.AluOpType.mult
    ADD = mybir.AluOpType.add

    with tile.TileContext(nc) as tc:
        with (
            tc.tile_pool(name="wpool", bufs=1) as wpool,
            tc.tile_pool(name="state", bufs=1) as state,
            tc.tile_pool(name="work", bufs=2) as work,
            tc.tile_pool(name="xg", bufs=3) as xgp,
            tc.tile_pool(name="ps", bufs=1, space="PSUM") as ps,
        ):
            def load(name, src, shape):
                t = wpool.tile(shape, dt, name=name)
                nc.sync.dma_start(out=t[:], in_=src[:])
                return t

            s_lpT = load("s_lpT", lpT, [128, 8 * ATT])
            s_w2T = load("s_w2T", w2T, [K, ATT])
            s_ewT = load("s_ewT", ewT, [ATT, 1])
            s_negc = load("s_negc", negc, [1, 1])
            s_ones = load("s_ones", onesrow, [1, 128])
            s_unmask = load("s_unmask", unmask, [1, BPC * T_ENC])
            s_maskT = load("s_maskT", maskT, [128, 2 * BPC])
            s_peT = load("s_peT", peT, [ATT, BPC * T_ENC])
            s_enc = load("s_enc", encT, [128, BPC * 2 * 4 * 128])
            s_w0cT = load("s_w0cT", w0cT, [128, 4 * NSLOT * 128])
            s_w1T = load("s_w1T", w1T, [128, 8 * NSLOT * 128])
            s_decwT = load("s_decwT", decwT, [128, 12 * (MEL + 1)])

            awc = state.tile([1, BPC * TPADDED], dt, name="awc")
            nc.vector.memset(awc[:], 0.0)
            im2col = state.tile([K, BPC * T_ENC], dt, name="im2col")
            nc.vector.memset(im2col[:], 0.0)
            h2 = state.tile([128, 16], dt, name="h2")
            nc.vector.memset(h2[:], 0.0)
            melbuf = state.tile([128, T_DEC * BPC], dt, name="melbuf")

            def step_body(i):
                # ---------- attention ----------
                qT = ps.tile([ATT, BPC], dt, name="qT", tag="psA")
                for hc in range(8):
                    nc.tensor.matmul(qT[:], s_lpT[:, hc * ATT:(hc + 1) * ATT],
                                     h2[:, 2 * hc:2 * hc + 2],
                                     start=(hc == 0), stop=(hc == 7))
                qTs = work.tile([ATT, BPC], dt, name="qTs")
                nc.vector.tensor_copy(qTs[:], qT[:])

                loc = ps.tile([ATT, BPC * T_ENC], dt, name="loc", tag="psB")
                nc.tensor.matmul(loc[:], s_w2T[:], im2col[:], start=True, stop=True)
                tmp = work.tile([ATT, BPC * T_ENC], dt, name="tmp")
                nc.vector.tensor_add(tmp[:], loc[:], s_peT[:])
                X = work.tile([ATT, BPC * T_ENC], dt, name="X")
                for b in range(BPC):
                    nc.scalar.activation(X[:, b * T_ENC:(b + 1) * T_ENC],
                                         tmp[:, b * T_ENC:(b + 1) * T_ENC],
                                         Tanh, bias=qTs[:, b:b + 1])

                enf = ps.tile([1, BPC * T_ENC], dt, name="enf", tag="psC")
                nc.tensor.matmul(enf[:], s_ewT[:], X[:], start=True, stop=False)
                nc.tensor.matmul(enf[:], s_negc[:], s_unmask[:], start=False,
                                 stop=True)
                mexp = work.tile([1, BPC * T_ENC], dt, name="mexp")
                denom = work.tile([1, BPC], dt, name="denom")
                for b in range(BPC):
                    nc.scalar.activation(mexp[:, b * T_ENC:(b + 1) * T_ENC],
                                         enf[:, b * T_ENC:(b + 1) * T_ENC],
                                         Exp, accum_out=denom[:, b:b + 1])
                recip = work.tile([1, BPC], dt, name="recip")
                nc.vector.reciprocal(recip[:], denom[:])
                # broadcast recip to 128 partitions via rank-1 matmul
                recipP = ps.tile([128, BPC], dt, name="recipP", tag="psE")
                nc.tensor.matmul(recipP[:], s_ones[:], recip[:], start=True,
                                 stop=True)
                recipPs = work.tile([128, BPC], dt, name="recipPs")
                nc.vector.tensor_copy(recipPs[:], recipP[:])

                enT = ps.tile([128, 2 * BPC], dt, name="enT", tag="psA")
                for c in range(2 * BPC):
                    nc.tensor.matmul(enT[:, c:c + 1], X[:, c * 128:(c + 1) * 128],
                                     s_ewT[:], start=True, stop=True)
                mexpT = work.tile([128, 2 * BPC], dt, name="mexpT")
                nc.scalar.activation(mexpT[:], enT[:], Exp)
                nc.vector.tensor_mul(mexpT[:], mexpT[:], s_maskT[:])

                ctxp = ps.tile([128, BPC * 4], dt, name="ctxp", tag="psD")
                for b in range(BPC):
                    for ec in range(4):
                        col = b * 4 + ec
                        for t2 in range(2):
                            off = ((b * 2 + t2) * 4 + ec) * 128
                            nc.tensor.matmul(
                                ctxp[:, col:col + 1], s_enc[:, off:off + 128],
                                mexpT[:, (b * 2 + t2):(b * 2 + t2) + 1],
                                start=(t2 == 0), stop=(t2 == 1))
                ctxT = work.tile([128, BPC * 4], dt, name="ctxT")
                for b in range(BPC):
                    nc.vector.tensor_mul(
                        ctxT[:, b * 4:(b + 1) * 4], ctxp[:, b * 4:(b + 1) * 4],
                        recipPs[:, b:b + 1].broadcast_to([128, 4]))

                # ---------- LSTM layer 0 ----------
                g0 = ps.tile([128, 2 * NSLOT], dt, name="g0", tag="psB")
                xgt = xgp.tile([128, 2 * NSLOT], dt, name="xgt")
                nc.sync.dma_start(out=xgt[:], in_=x0g[bass.ts(i, 1), :, :])
                for s in range(NSLOT):
                    for ec in range(4):
                        woff = (ec * NSLOT + s) * 128
                        rhs = bass.AP(tensor=ctxT.tensor,
                                      offset=ctxT.offset + ec,
                                      ap=[ctxT.ap[0], [4, BPC]])
                        nc.tensor.matmul(g0[:, 2 * s:2 * s + 2],
                                         s_w0cT[:, woff:woff + 128], rhs,
                                         start=(ec == 0), stop=(ec == 3))
                T0 = work.tile([128, 2 * NSLOT], dt, name="T0")
                nc.vector.tensor_add(T0[:], g0[:], xgt[:])
                nc.scalar.activation(T0[:], T0[:], Tanh)
                # c2 = (ti+1)*tg ; tc = tanh(0.5*c2) ; h1' = (to+1)*tc = 2*h1
                c2 = work.tile([128, 16], dt, name="c2")
                nc.vector.scalar_tensor_tensor(c2[:], T0[:, 0:16], 1.0,
                                               T0[:, 32:48], ADD, MUL)
                nc.scalar.activation(c2[:], c2[:], Tanh, scale=0.5)
                h1 = work.tile([128, 16], dt, name="h1")
                nc.vector.scalar_tensor_tensor(h1[:], T0[:, 16:32], 1.0,
                                               c2[:], ADD, MUL)

                # ---------- LSTM layer 1 ----------
                g1 = ps.tile([128, 2 * NSLOT], dt, name="g1", tag="psC")
                for s in range(NSLOT):
                    for hc in range(8):
                        woff = (hc * NSLOT + s) * 128
                        nc.tensor.matmul(g1[:, 2 * s:2 * s + 2],
                                         s_w1T[:, woff:woff + 128],
                                         h1[:, 2 * hc:2 * hc + 2],
                                         start=(hc == 0), stop=(hc == 7))
                T1 = work.tile([128, 2 * NSLOT], dt, name="T1")
                nc.scalar.activation(T1[:], g1[:], Tanh)
                c3 = work.tile([128, 16], dt, name="c3")
                nc.vector.scalar_tensor_tensor(c3[:], T1[:, 0:16], 1.0,
                                               T1[:, 32:48], ADD, MUL)
                nc.scalar.activation(c3[:], c3[:], Tanh, scale=0.5)
                nc.vector.scalar_tensor_tensor(h2[:], T1[:, 16:32], 1.0,
                                               c3[:], ADD, MUL)

                # ---------- projection ----------
                melp = ps.tile([MEL + 1, BPC], dt, name="melp", tag="psD")
                for kc in range(12):
                    woff = kc * (MEL + 1)
                    if kc < 8:
                        rhs = h2[:, 2 * kc:2 * kc + 2]
                    else:
                        ec = kc - 8
                        rhs = bass.AP(tensor=ctxT.tensor,
                                      offset=ctxT.offset + ec,
                                      ap=[ctxT.ap[0], [4, BPC]])
                    nc.tensor.matmul(melp[:], s_decwT[:, woff:woff + MEL + 1],
                                     rhs, start=(kc == 0), stop=(kc == 11))
                nc.vector.tensor_copy(melbuf[:MEL + 1, bass.ts(i, BPC)], melp[:])

                # ---------- awc / im2col update for next step ----------
                aw = work.tile([1, BPC * T_ENC], dt, name="aw")
                for b in range(BPC):
                    nc.vector.tensor_mul(
                        aw[:, b * T_ENC:(b + 1) * T_ENC],
                        mexp[:, b * T_ENC:(b + 1) * T_ENC],
                        recip[:, b:b + 1].broadcast_to([1, T_ENC]))
                awc_v = bass.AP(tensor=awc.tensor, offset=awc.offset + PAD,
                                ap=[awc.ap[0], [TPADDED, BPC], [1, T_ENC]])
                nc.vector.tensor_add(
                    awc_v, awc_v, aw[:].rearrange("p (b t) -> p b t", b=BPC))
                for b in range(BPC):
                    src = bass.AP(tensor=awc.tensor,
                                  offset=awc.offset + b * TPADDED,
                                  ap=[awc.ap[0], [1, K], [1, T_ENC]])
                    dst = bass.AP(tensor=im2col.tensor,
                                  offset=im2col.offset + b * T_ENC,
                                  ap=[im2col.ap[0], [1, T_ENC]])
                    nc.sync.dma_start(out=dst, in_=src)

            with tc.For_i(0, T_DEC, 1,
                          hint_engines=(mybir.EngineType.PE,)) as i:
                step_body(i)

            nc.sync.dma_start(out=out_ext[:], in_=melbuf[:MEL + 1, :])

    nc.compile()
    return nc


def _prep_inputs(inputs):
    f32 = np.float32
    enc = np.asarray(inputs["encoder_output"], f32)
    mels = np.asarray(inputs["padded_mels"], f32)
    tl = np.asarray(inputs["text_lengths"]).astype(np.int64)

    enc_proj_w = np.asarray(inputs["enc_proj_w"], f32)
    lstm_proj_w = np.asarray(inputs["lstm_proj_w"], f32)
    loc_conv_w = np.asarray(inputs["loc_conv_w"], f32)
    loc_conv_b = np.asarray(inputs["loc_conv_b"], f32)
    loc_dense_w = np.asarray(inputs["loc_dense_w"], f32)
    loc_dense_b = np.asarray(inputs["loc_dense_b"], f32)
    e_w = np.asarray(inputs["e_w"], f32)
    e_b = np.asarray(inputs["e_b"], f32)
    prenet1_w = np.asarray(inputs["prenet1_w"], f32)
    prenet2_w = np.asarray(inputs["prenet2_w"], f32)
    w_ih0 = np.asarray(inputs["w_ih0"], f32)
    b_ih0 = np.asarray(inputs["b_ih0"], f32)
    b_hh0 = np.asarray(inputs["b_hh0"], f32)
    w_ih1 = np.asarray(inputs["w_ih1"], f32)
    b_ih1 = np.asarray(inputs["b_ih1"], f32)
    b_hh1 = np.asarray(inputs["b_hh1"], f32)
    proj_w = np.asarray(inputs["proj_w"], f32)
    proj_b = np.asarray(inputs["proj_b"], f32)
    stop_w = np.asarray(inputs["stop_w"], f32)
    stop_b = np.asarray(inputs["stop_b"], f32)

    # keep (i, o, g) rows, drop dead f; scale i,o rows by 0.5 (sigmoid trick)
    def iog(w):
        wi, wf, wg, wo = np.split(w, 4, axis=0)
        return np.concatenate([wi * 0.5, wo * 0.5, wg], axis=0)

    w0 = iog(w_ih0)
    b0 = iog((b_ih0 + b_hh0)[:, None])[:, 0]
    w1 = iog(w_ih1) * 0.5          # extra 0.5: consumes h1' = 2*h1
    b1 = iog((b_ih1 + b_hh1)[:, None])[:, 0]
    assert np.abs(b1).max() == 0.0, "nonzero LSTM1 bias not supported"
    w0x, w0c = w0[:, :PRE], w0[:, PRE:]

    prev = np.concatenate([np.zeros((B, 1, MEL), f32), mels[:, :-1]], axis=1)
    x = np.maximum(prev @ prenet1_w.T, 0.0)
    x = np.maximum(x @ prenet2_w.T, 0.0)
    x0gates = x @ w0x.T + b0                     # [16, 200, 3072]

    W2 = loc_dense_w @ loc_conv_w[:, 0, :]       # [128, 31]
    pe_bias = loc_dense_b + loc_dense_w @ loc_conv_b
    pe = enc @ enc_proj_w.T + pe_bias            # [16, 256, 128]
    mask = (np.arange(T_ENC)[None, :] < tl[:, None])

    # en = X . e_w + e_b : fold e_b into the mask matmul? simpler: e_b is a
    # constant added to every energy -> softmax invariant; drop it.
    del e_b

    decw_h = 0.5 * np.concatenate([proj_w[:, :H], stop_w[:, :H]], axis=0)
    decw_c = np.concatenate([proj_w[:, H:], stop_w[:, H:]], axis=0)
    decb = np.concatenate([proj_b, stop_b], axis=0)
    assert np.abs(decb).max() == 0.0, "nonzero proj bias not supported"
    decw = np.concatenate([decw_h, decw_c], axis=1)   # [81, 1536]

    lpTm = (0.5 * lstm_proj_w).T                 # [1024, 128]

    base = {
        "lpT": np.ascontiguousarray(
            lpTm.reshape(8, 128, ATT).transpose(1, 0, 2).reshape(128, 8 * ATT)),
        "w2T": np.ascontiguousarray(W2.T),
        "ewT": np.ascontiguousarray(e_w.T),
        "negc": np.full((1, 1), NEG, f32),
        "onesrow": np.ones((1, 128), f32),
        "w0cT": np.ascontiguousarray(
            w0c.T.reshape(4, 128, NSLOT, 128).transpose(1, 0, 2, 3)
            .reshape(128, 4 * NSLOT * 128)),
        "w1T": np.ascontiguousarray(
            w1.T.reshape(8, 128, NSLOT, 128).transpose(1, 0, 2, 3)
            .reshape(128, 8 * NSLOT * 128)),
        "decwT": np.ascontiguousarray(
            decw.T.reshape(12, 128, MEL + 1).transpose(1, 0, 2)
            .reshape(128, 12 * (MEL + 1))),
    }
    in_maps = []
    for c in range(NCORES):
        bs = slice(c * BPC, (c + 1) * BPC)
        m = dict(base)
        mk = mask[bs]
        m["unmask"] = np.ascontiguousarray(
            (~mk).astype(f32).reshape(1, BPC * T_ENC))
        m["maskT"] = np.ascontiguousarray(
            mk.reshape(BPC, 2, 128).transpose(2, 0, 1).reshape(128, BPC * 2)
            .astype(f32))
        m["peT"] = np.ascontiguousarray(
            pe[bs].reshape(BPC * T_ENC, ATT).T)
        m["encT"] = np.ascontiguousarray(
            enc[bs].reshape(BPC, 2, 128, 4, 128).transpose(2, 0, 1, 3, 4)
            .reshape(128, BPC * 2 * 4 * 128))
        xg = x0gates[bs].transpose(1, 2, 0)      # [200, 3072, 2]
        m["x0g"] = np.ascontiguousarray(
            xg.reshape(T_DEC, NSLOT, 128, BPC).transpose(0, 2, 1, 3)
            .reshape(T_DEC, 128, 2 * NSLOT))
        in_maps.append(m)
    return in_maps


def kernel(**inputs):
    if "nc" not in _cache:
        _cache["nc"] = _build_program()
    nc = _cache["nc"]
    in_maps = _prep_inputs(inputs)
    res = run_bass_kernel_spmd(nc, in_maps, core_ids=list(range(NCORES)))
    mels = np.zeros((B, T_DEC, MEL), np.float32)
    stops = np.zeros((B, T_DEC), np.float32)
    for c in range(NCORES):
        o = res.results[c]["out"].reshape(MEL + 1, T_DEC, BPC)
        for b in range(BPC):
            mels[c * BPC + b] = o[:MEL, :, b].T
            stops[c * BPC + b] = o[MEL, :, b]
    return mels, stops


# revision 5
# speedup vs baseline: 1.9654x; 1.9654x over previous
"""Trainium2 Bass kernel for nn_Decoder (Tacotron-style decoder, 200-step scan).

Strategy: data-parallel over batch (B=16 -> 8 cores x 2 sequences each); all
weights resident in SBUF; one For_i hardware loop over the 200 decoder steps.
The LSTM state resets every step, so the only recurrent state is the
cumulative attention weights (awc) and h2. The forget gate is dead
(c_prev == 0) and is dropped. sigmoid(x) = 0.5*tanh(x/2)+0.5 so tanh+exp (one
ACT table set) cover all nonlinearities; scale factors are folded into
weights on the host (i/o gate rows x0.5; h' = 2h absorbed into consumers).

Host precomputes: prenet + x-part of LSTM0 gates for all steps, proc_enc,
the fused location conv+dense kernel W2 = loc_dense_w @ loc_conv_w, masks.
"""
import sys
sys.path.insert(0, "/opt/trn_rl_repo")
import numpy as np
import ml_dtypes

import concourse.bass as bass
import concourse.tile as tile
from concourse import bacc, mybir
from concourse.bass_utils import run_bass_kernel_spmd

B, T_ENC, T_DEC = 16, 256, 200
E, ATT, PRE, H, MEL, LOC, K = 512, 128, 256, 1024, 80, 32, 31
NCORES = 8
BPC = B // NCORES          # 2 sequences per core
G3 = 3 * H                 # 3072 gates per layer (i, o, g)
NSLOT = G3 // 128          # 24
PAD = (K - 1) // 2         # 15
TPADDED = T_ENC + 2 * PAD  # 286
NEG = -30.0                # mask offset inside exp

F32 = mybir.dt.float32
BF16 = mybir.dt.bfloat16
NPBF = ml_dtypes.bfloat16
_cache = {}


def _build_program():
    nc = bacc.Bacc("TRN2", target_bir_lowering=False, debug=False,
                   enable_asserts=False, num_devices=NCORES)
    dt = F32

    def din(name, shape, ddt=F32):
        return nc.dram_tensor(name, list(shape), ddt, kind="ExternalInput").ap()

    # all inputs pre-laid-out on host to match SBUF [partition, free] exactly
    lpT = din("lpT", [128, 8 * ATT], BF16)
    w2T = din("w2T", [K, ATT])
    ewT = din("ewT", [ATT, 1], BF16)
    negc = din("negc", [1, 1], BF16)
    onesrow = din("onesrow", [1, 128])
    unmask = din("unmask", [1, BPC * T_ENC], BF16)
    maskT = din("maskT", [128, 2 * BPC], BF16)
    peT = din("peT", [ATT, BPC * T_ENC], BF16)
    encT = din("encT", [128, BPC * 2 * 4 * 128], BF16)
    w0cT = din("w0cT", [128, 4 * NSLOT * 128], BF16)
    w1T = din("w1T", [128, 8 * NSLOT * 128], BF16)
    decwT = din("decwT", [128, 12 * (MEL + 1)], BF16)
    x0g = din("x0g", [T_DEC, 128, 2 * NSLOT], BF16)
    out_ext = nc.dram_tensor("out", [MEL + 1, T_DEC * BPC], dt,
                             kind="ExternalOutput").ap()

    Tanh = mybir.ActivationFunctionType.Tanh
    Exp = mybir.ActivationFunctionType.Exp
    MUL = mybir.AluOpType.mult
    ADD = mybir.AluOpType.add

    with tile.TileContext(nc) as tc:
        with (
            tc.tile_pool(name="wpool", bufs=1) as wpool,
            tc.tile_pool(name="state", bufs=1) as state,
            tc.tile_pool(name="work", bufs=2) as work,
            tc.tile_pool(name="xg", bufs=3) as xgp,
            tc.tile_pool(name="ps", bufs=1, space="PSUM") as ps,
        ):
            def load(name, src, shape, ldt=F32):
                t = wpool.tile(shape, ldt, name=name)
                nc.sync.dma_start(out=t[:], in_=src[:])
                return t

            s_lpT = load("s_lpT", lpT, [128, 8 * ATT], BF16)
            s_w2T = load("s_w2T", w2T, [K, ATT])
            s_ewT = load("s_ewT", ewT, [ATT, 1], BF16)
            s_negc = load("s_negc", negc, [1, 1], BF16)
            s_ones = load("s_ones", onesrow, [1, 128])
            s_unmask = load("s_unmask", unmask, [1, BPC * T_ENC], BF16)
            s_maskT = load("s_maskT", maskT, [128, 2 * BPC], BF16)
            s_peT = load("s_peT", peT, [ATT, BPC * T_ENC], BF16)
            s_enc = load("s_enc", encT, [128, BPC * 2 * 4 * 128], BF16)
            s_w0cT = load("s_w0cT", w0cT, [128, 4 * NSLOT * 128], BF16)
            s_w1T = load("s_w1T", w1T, [128, 8 * NSLOT * 128], BF16)
            s_decwT = load("s_decwT", decwT, [128, 12 * (MEL + 1)], BF16)

            awc = state.tile([1, BPC * TPADDED], dt, name="awc")
            nc.vector.memset(awc[:], 0.0)
            im2col = state.tile([K, BPC * T_ENC], dt, name="im2col")
            nc.vector.memset(im2col[:], 0.0)
            h2 = state.tile([128, 16], BF16, name="h2")
            nc.vector.memset(h2[:], 0.0)
            melbuf = state.tile([128, T_DEC * BPC], dt, name="melbuf")

            def step_body(i):
                # ---------- attention ----------
                qT = ps.tile([ATT, BPC], dt, name="qT", tag="psA")
                for hc in range(8):
                    nc.tensor.matmul(qT[:], s_lpT[:, hc * ATT:(hc + 1) * ATT],
                                     h2[:, 2 * hc:2 * hc + 2],
                                     start=(hc == 0), stop=(hc == 7))
                qTs = work.tile([ATT, BPC], dt, name="qTs")
                nc.vector.tensor_copy(qTs[:], qT[:])

                loc = ps.tile([ATT, BPC * T_ENC], dt, name="loc", tag="psB")
                nc.tensor.matmul(loc[:], s_w2T[:], im2col[:], start=True, stop=True)
                tmp = work.tile([ATT, BPC * T_ENC], dt, name="tmp")
                nc.vector.tensor_add(tmp[:], loc[:], s_peT[:])
                X = work.tile([ATT, BPC * T_ENC], BF16, name="X")
                for b in range(BPC):
                    nc.scalar.activation(X[:, b * T_ENC:(b + 1) * T_ENC],
                                         tmp[:, b * T_ENC:(b + 1) * T_ENC],
                                         Tanh, bias=qTs[:, b:b + 1])

                enf = ps.tile([1, BPC * T_ENC], dt, name="enf", tag="psC")
                nc.tensor.matmul(enf[:], s_ewT[:], X[:], start=True, stop=False)
                nc.tensor.matmul(enf[:], s_negc[:], s_unmask[:], start=False,
                                 stop=True)
                mexp = work.tile([1, BPC * T_ENC], dt, name="mexp")
                denom = work.tile([1, BPC], dt, name="denom")
                for b in range(BPC):
                    nc.scalar.activation(mexp[:, b * T_ENC:(b + 1) * T_ENC],
                                         enf[:, b * T_ENC:(b + 1) * T_ENC],
                                         Exp, accum_out=denom[:, b:b + 1])
                recip = work.tile([1, BPC], dt, name="recip")
                nc.vector.reciprocal(recip[:], denom[:])
                # broadcast recip to 128 partitions via rank-1 matmul
                recipP = ps.tile([128, BPC], dt, name="recipP", tag="psE")
                nc.tensor.matmul(recipP[:], s_ones[:], recip[:], start=True,
                                 stop=True)
                recipPs = work.tile([128, BPC], dt, name="recipPs")
                nc.vector.tensor_copy(recipPs[:], recipP[:])

                enT = ps.tile([128, 2 * BPC], dt, name="enT", tag="psA")
                for c in range(2 * BPC):
                    nc.tensor.matmul(enT[:, c:c + 1], X[:, c * 128:(c + 1) * 128],
                                     s_ewT[:], start=True, stop=True)
                mexpT = work.tile([128, 2 * BPC], BF16, name="mexpT")
                nc.scalar.activation(mexpT[:], enT[:], Exp)
                nc.vector.tensor_mul(mexpT[:], mexpT[:], s_maskT[:])

                ctxp = ps.tile([128, BPC * 4], dt, name="ctxp", tag="psD")
                for b in range(BPC):
                    for ec in range(4):
                        col = b * 4 + ec
                        for t2 in range(2):
                            off = ((b * 2 + t2) * 4 + ec) * 128
                            nc.tensor.matmul(
                                ctxp[:, col:col + 1], s_enc[:, off:off + 128],
                                mexpT[:, (b * 2 + t2):(b * 2 + t2) + 1],
                                start=(t2 == 0), stop=(t2 == 1))
                ctxT = work.tile([128, BPC * 4], BF16, name="ctxT")
                for b in range(BPC):
                    nc.vector.tensor_mul(
                        ctxT[:, b * 4:(b + 1) * 4], ctxp[:, b * 4:(b + 1) * 4],
                        recipPs[:, b:b + 1].broadcast_to([128, 4]))

                # ---------- LSTM layer 0 ----------
                g0 = ps.tile([128, 2 * NSLOT], dt, name="g0", tag="psB")
                xgt = xgp.tile([128, 2 * NSLOT], BF16, name="xgt")
                nc.sync.dma_start(out=xgt[:], in_=x0g[bass.ts(i, 1), :, :])
                for s in range(NSLOT):
                    for ec in range(4):
                        woff = (ec * NSLOT + s) * 128
                        rhs = bass.AP(tensor=ctxT.tensor,
                                      offset=ctxT.offset + ec,
                                      ap=[ctxT.ap[0], [4, BPC]])
                        nc.tensor.matmul(g0[:, 2 * s:2 * s + 2],
                                         s_w0cT[:, woff:woff + 128], rhs,
                                         start=(ec == 0), stop=(ec == 3))
                T0 = work.tile([128, 2 * NSLOT], dt, name="T0")
                nc.vector.tensor_add(T0[:], g0[:], xgt[:])
                nc.scalar.activation(T0[:], T0[:], Tanh)
                # c2 = (ti+1)*tg ; tc = tanh(0.5*c2) ; h1' = (to+1)*tc = 2*h1
                c2 = work.tile([128, 16], dt, name="c2")
                nc.vector.scalar_tensor_tensor(c2[:], T0[:, 0:16], 1.0,
                                               T0[:, 32:48], ADD, MUL)
                nc.scalar.activation(c2[:], c2[:], Tanh, scale=0.5)
                h1 = work.tile([128, 16], BF16, name="h1")
                nc.vector.scalar_tensor_tensor(h1[:], T0[:, 16:32], 1.0,
                                               c2[:], ADD, MUL)

                # ---------- LSTM layer 1 ----------
                g1 = ps.tile([128, 2 * NSLOT], dt, name="g1", tag="psC")
                for s in range(NSLOT):
                    for hc in range(8):
                        woff = (hc * NSLOT + s) * 128
                        nc.tensor.matmul(g1[:, 2 * s:2 * s + 2],
                                         s_w1T[:, woff:woff + 128],
                                         h1[:, 2 * hc:2 * hc + 2],
                                         start=(hc == 0), stop=(hc == 7))
                T1 = work.tile([128, 2 * NSLOT], dt, name="T1")
                nc.scalar.activation(T1[:], g1[:], Tanh)
                c3 = work.tile([128, 16], dt, name="c3")
                nc.vector.scalar_tensor_tensor(c3[:], T1[:, 0:16], 1.0,
                                               T1[:, 32:48], ADD, MUL)
                nc.scalar.activation(c3[:], c3[:], Tanh, scale=0.5)
                nc.vector.scalar_tensor_tensor(h2[:], T1[:, 16:32], 1.0,
                                               c3[:], ADD, MUL)

                # ---------- projection ----------
                melp = ps.tile([MEL + 1, BPC], dt, name="melp", tag="psD")
                for kc in range(12):
                    woff = kc * (MEL + 1)
                    if kc < 8:
                        rhs = h2[:, 2 * kc:2 * kc + 2]
                    else:
                        ec = kc - 8
                        rhs = bass.AP(tensor=ctxT.tensor,
                                      offset=ctxT.offset + ec,
                                      ap=[ctxT.ap[0], [4, BPC]])
                    nc.tensor.matmul(melp[:], s_decwT[:, woff:woff + MEL + 1],
                                     rhs, start=(kc == 0), stop=(kc == 11))
                nc.vector.tensor_copy(melbuf[:MEL + 1, bass.ts(i, BPC)], melp[:])

                # ---------- awc / im2col update for next step ----------
                aw = work.tile([1, BPC * T_ENC], dt, name="aw")
                for b in range(BPC):
                    nc.vector.tensor_mul(
                        aw[:, b * T_ENC:(b + 1) * T_ENC],
                        mexp[:, b * T_ENC:(b + 1) * T_ENC],
                        recip[:, b:b + 1].broadcast_to([1, T_ENC]))
                awc_v = bass.AP(tensor=awc.tensor, offset=awc.offset + PAD,
                                ap=[awc.ap[0], [TPADDED, BPC], [1, T_ENC]])
                nc.vector.tensor_add(
                    awc_v, awc_v, aw[:].rearrange("p (b t) -> p b t", b=BPC))
                for b in range(BPC):
                    src = bass.AP(tensor=awc.tensor,
                                  offset=awc.offset + b * TPADDED,
                                  ap=[awc.ap[0], [1, K], [1, T_ENC]])
                    dst = bass.AP(tensor=im2col.tensor,
                                  offset=im2col.offset + b * T_ENC,
                                  ap=[im2col.ap[0], [1, T_ENC]])
                    nc.sync.dma_start(out=dst, in_=src)

            with tc.For_i(0, T_DEC, 1,
                          hint_engines=(mybir.EngineType.PE,)) as i:
                step_body(i)

            nc.sync.dma_start(out=out_ext[:], in_=melbuf[:MEL + 1, :])

    nc.compile()
    return nc


def _prep_inputs(inputs):
    f32 = np.float32
    enc = np.asarray(inputs["encoder_output"], f32)
    mels = np.asarray(inputs["padded_mels"], f32)
    tl = np.asarray(inputs["text_lengths"]).astype(np.int64)

    enc_proj_w = np.asarray(inputs["enc_proj_w"], f32)
    lstm_proj_w = np.asarray(inputs["lstm_proj_w"], f32)
    loc_conv_w = np.asarray(inputs["loc_conv_w"], f32)
    loc_conv_b = np.asarray(inputs["loc_conv_b"], f32)
    loc_dense_w = np.asarray(inputs["loc_dense_w"], f32)
    loc_dense_b = np.asarray(inputs["loc_dense_b"], f32)
    e_w = np.asarray(inputs["e_w"], f32)
    e_b = np.asarray(inputs["e_b"], f32)
    prenet1_w = np.asarray(inputs["prenet1_w"], f32)
    prenet2_w = np.asarray(inputs["prenet2_w"], f32)
    w_ih0 = np.asarray(inputs["w_ih0"], f32)
    b_ih0 = np.asarray(inputs["b_ih0"], f32)
    b_hh0 = np.asarray(inputs["b_hh0"], f32)
    w_ih1 = np.asarray(inputs["w_ih1"], f32)
    b_ih1 = np.asarray(inputs["b_ih1"], f32)
    b_hh1 = np.asarray(inputs["b_hh1"], f32)
    proj_w = np.asarray(inputs["proj_w"], f32)
    proj_b = np.asarray(inputs["proj_b"], f32)
    stop_w = np.asarray(inputs["stop_w"], f32)
    stop_b = np.asarray(inputs["stop_b"], f32)

    # keep (i, o, g) rows, drop dead f; scale i,o rows by 0.5 (sigmoid trick)
    def iog(w):
        wi, wf, wg, wo = np.split(w, 4, axis=0)
        return np.concatenate([wi * 0.5, wo * 0.5, wg], axis=0)

    w0 = iog(w_ih0)
    b0 = iog((b_ih0 + b_hh0)[:, None])[:, 0]
    w1 = iog(w_ih1) * 0.5          # extra 0.5: consumes h1' = 2*h1
    b1 = iog((b_ih1 + b_hh1)[:, None])[:, 0]
    assert np.abs(b1).max() == 0.0, "nonzero LSTM1 bias not supported"
    w0x, w0c = w0[:, :PRE], w0[:, PRE:]

    prev = np.concatenate([np.zeros((B, 1, MEL), f32), mels[:, :-1]], axis=1)
    x = np.maximum(prev @ prenet1_w.T, 0.0)
    x = np.maximum(x @ prenet2_w.T, 0.0)
    x0gates = x @ w0x.T + b0                     # [16, 200, 3072]

    W2 = loc_dense_w @ loc_conv_w[:, 0, :]       # [128, 31]
    pe_bias = loc_dense_b + loc_dense_w @ loc_conv_b
    pe = enc @ enc_proj_w.T + pe_bias            # [16, 256, 128]
    mask = (np.arange(T_ENC)[None, :] < tl[:, None])

    # en = X . e_w + e_b : fold e_b into the mask matmul? simpler: e_b is a
    # constant added to every energy -> softmax invariant; drop it.
    del e_b

    decw_h = 0.5 * np.concatenate([proj_w[:, :H], stop_w[:, :H]], axis=0)
    decw_c = np.concatenate([proj_w[:, H:], stop_w[:, H:]], axis=0)
    decb = np.concatenate([proj_b, stop_b], axis=0)
    assert np.abs(decb).max() == 0.0, "nonzero proj bias not supported"
    decw = np.concatenate([decw_h, decw_c], axis=1)   # [81, 1536]

    lpTm = (0.5 * lstm_proj_w).T                 # [1024, 128]

    base = {
        "lpT": np.ascontiguousarray(
            lpTm.reshape(8, 128, ATT).transpose(1, 0, 2).reshape(128, 8 * ATT)
            .astype(NPBF)),
        "w2T": np.ascontiguousarray(W2.T),
        "ewT": np.ascontiguousarray(e_w.T.astype(NPBF)),
        "negc": np.full((1, 1), NEG, NPBF),
        "onesrow": np.ones((1, 128), f32),
        "w0cT": np.ascontiguousarray(
            w0c.T.reshape(4, 128, NSLOT, 128).transpose(1, 0, 2, 3)
            .reshape(128, 4 * NSLOT * 128).astype(NPBF)),
        "w1T": np.ascontiguousarray(
            w1.T.reshape(8, 128, NSLOT, 128).transpose(1, 0, 2, 3)
            .reshape(128, 8 * NSLOT * 128).astype(NPBF)),
        "decwT": np.ascontiguousarray(
            decw.T.reshape(12, 128, MEL + 1).transpose(1, 0, 2)
            .reshape(128, 12 * (MEL + 1)).astype(NPBF)),
    }
    in_maps = []
    for c in range(NCORES):
        bs = slice(c * BPC, (c + 1) * BPC)
        m = dict(base)
        mk = mask[bs]
        m["unmask"] = np.ascontiguousarray(
            (~mk).astype(NPBF).reshape(1, BPC * T_ENC))
        m["maskT"] = np.ascontiguousarray(
            mk.reshape(BPC, 2, 128).transpose(2, 0, 1).reshape(128, BPC * 2)
            .astype(NPBF))
        m["peT"] = np.ascontiguousarray(
            pe[bs].reshape(BPC * T_ENC, ATT).T.astype(NPBF))
        m["encT"] = np.ascontiguousarray(
            enc[bs].reshape(BPC, 2, 128, 4, 128).transpose(2, 0, 1, 3, 4)
            .reshape(128, BPC * 2 * 4 * 128).astype(NPBF))
        xg = x0gates[bs].transpose(1, 2, 0)      # [200, 3072, 2]
        m["x0g"] = np.ascontiguousarray(
            xg.reshape(T_DEC, NSLOT, 128, BPC).transpose(0, 2, 1, 3)
            .reshape(T_DEC, 128, 2 * NSLOT).astype(NPBF))
        in_maps.append(m)
    return in_maps


def kernel(**inputs):
    if "nc" not in _cache:
        _cache["nc"] = _build_program()
    nc = _cache["nc"]
    in_maps = _prep_inputs(inputs)
    res = run_bass_kernel_spmd(nc, in_maps, core_ids=list(range(NCORES)))
    mels = np.zeros((B, T_DEC, MEL), np.float32)
    stops = np.zeros((B, T_DEC), np.float32)
    for c in range(NCORES):
        o = res.results[c]["out"].reshape(MEL + 1, T_DEC, BPC)
        for b in range(BPC):
            mels[c * BPC + b] = o[:MEL, :, b].T
            stops[c * BPC + b] = o[MEL, :, b]
    return mels, stops


# revision 6
# speedup vs baseline: 2.1010x; 1.0690x over previous
"""Trainium2 Bass kernel for nn_Decoder (Tacotron-style decoder, 200-step scan).

Strategy: data-parallel over batch (B=16 -> 8 cores x 2 sequences each); all
weights resident in SBUF; one For_i hardware loop over the 200 decoder steps.
The LSTM state resets every step, so the only recurrent state is the
cumulative attention weights (awc) and h2. The forget gate is dead
(c_prev == 0) and is dropped. sigmoid(x) = 0.5*tanh(x/2)+0.5 so tanh+exp (one
ACT table set) cover all nonlinearities; scale factors are folded into
weights on the host (i/o gate rows x0.5; h' = 2h absorbed into consumers).

Host precomputes: prenet + x-part of LSTM0 gates for all steps, proc_enc,
the fused location conv+dense kernel W2 = loc_dense_w @ loc_conv_w, masks.
"""
import sys
sys.path.insert(0, "/opt/trn_rl_repo")
import numpy as np
import ml_dtypes

import concourse.bass as bass
import concourse.tile as tile
from concourse import bacc, mybir
from concourse.bass_utils import run_bass_kernel_spmd

B, T_ENC, T_DEC = 16, 256, 200
E, ATT, PRE, H, MEL, LOC, K = 512, 128, 256, 1024, 80, 32, 31
NCORES = 8
BPC = B // NCORES          # 2 sequences per core
G3 = 3 * H                 # 3072 gates per layer (i, o, g)
NSLOT = G3 // 128          # 24
PAD = (K - 1) // 2         # 15
TPADDED = T_ENC + 2 * PAD  # 286
NEG = -30.0                # mask offset inside exp

F32 = mybir.dt.float32
BF16 = mybir.dt.bfloat16
NPBF = ml_dtypes.bfloat16
_cache = {}


def _build_program():
    nc = bacc.Bacc("TRN2", target_bir_lowering=False, debug=False,
                   enable_asserts=False, num_devices=NCORES)
    dt = F32

    def din(name, shape, ddt=F32):
        return nc.dram_tensor(name, list(shape), ddt, kind="ExternalInput").ap()

    # all inputs pre-laid-out on host to match SBUF [partition, free] exactly
    lpT = din("lpT", [128, 8 * ATT], BF16)
    w2T = din("w2T", [K, ATT])
    ewT = din("ewT", [ATT, 1], BF16)
    negc = din("negc", [1, 1], BF16)
    onesrow = din("onesrow", [1, 128])
    unmask = din("unmask", [1, BPC * T_ENC], BF16)
    maskT = din("maskT", [128, 2 * BPC], BF16)
    peT = din("peT", [ATT, BPC * T_ENC], BF16)
    encT = din("encT", [128, BPC * 2 * 4 * 128], BF16)
    w0cT = din("w0cT", [128, 4 * NSLOT * 128], BF16)
    w1T = din("w1T", [128, 8 * NSLOT * 128], BF16)
    decwT = din("decwT", [128, 12 * (MEL + 1)], BF16)
    xT = din("xT", [128, 2 * T_DEC * BPC], BF16)
    w0xT = din("w0xT", [128, 2 * NSLOT * 128], BF16)
    out_ext = nc.dram_tensor("out", [MEL + 1, T_DEC * BPC], dt,
                             kind="ExternalOutput").ap()

    Tanh = mybir.ActivationFunctionType.Tanh
    Exp = mybir.ActivationFunctionType.Exp
    MUL = mybir.AluOpType.mult
    ADD = mybir.AluOpType.add

    with tile.TileContext(nc) as tc:
        with (
            tc.tile_pool(name="wpool", bufs=1) as wpool,
            tc.tile_pool(name="state", bufs=1) as state,
            tc.tile_pool(name="work", bufs=2) as work,
            tc.tile_pool(name="ps", bufs=1, space="PSUM") as ps,
        ):
            def load(name, src, shape, ldt=F32):
                t = wpool.tile(shape, ldt, name=name)
                nc.sync.dma_start(out=t[:], in_=src[:])
                return t

            s_lpT = load("s_lpT", lpT, [128, 8 * ATT], BF16)
            s_w2T = load("s_w2T", w2T, [K, ATT])
            s_ewT = load("s_ewT", ewT, [ATT, 1], BF16)
            s_negc = load("s_negc", negc, [1, 1], BF16)
            s_ones = load("s_ones", onesrow, [1, 128])
            s_unmask = load("s_unmask", unmask, [1, BPC * T_ENC], BF16)
            s_maskT = load("s_maskT", maskT, [128, 2 * BPC], BF16)
            s_peT = load("s_peT", peT, [ATT, BPC * T_ENC], BF16)
            s_enc = load("s_enc", encT, [128, BPC * 2 * 4 * 128], BF16)
            s_w0cT = load("s_w0cT", w0cT, [128, 4 * NSLOT * 128], BF16)
            s_w1T = load("s_w1T", w1T, [128, 8 * NSLOT * 128], BF16)
            s_decwT = load("s_decwT", decwT, [128, 12 * (MEL + 1)], BF16)
            s_xT = load("s_xT", xT, [128, 2 * T_DEC * BPC], BF16)
            s_w0xT = load("s_w0xT", w0xT, [128, 2 * NSLOT * 128], BF16)

            awc = state.tile([1, BPC * TPADDED], dt, name="awc")
            nc.vector.memset(awc[:], 0.0)
            im2col = state.tile([K, BPC * T_ENC], dt, name="im2col")
            nc.vector.memset(im2col[:], 0.0)
            h2 = state.tile([128, 16], BF16, name="h2")
            nc.vector.memset(h2[:], 0.0)
            melbuf = state.tile([128, T_DEC * BPC], dt, name="melbuf")

            # one-time: x-part of LSTM0 gates for all steps -> SBUF (bf16)
            NTB = T_DEC * BPC
            x0gs = state.tile([128, NSLOT * NTB], BF16, name="x0gs")
            for s_ in range(NSLOT):
                gp = ps.tile([128, NTB], dt, name="gp", tag="psB")
                for pc in range(2):
                    nc.tensor.matmul(gp[:],
                                     s_w0xT[:, (pc * NSLOT + s_) * 128:
                                            (pc * NSLOT + s_) * 128 + 128],
                                     s_xT[:, pc * NTB:(pc + 1) * NTB],
                                     start=(pc == 0), stop=(pc == 1))
                nc.vector.tensor_copy(x0gs[:, s_ * NTB:(s_ + 1) * NTB], gp[:])

            def step_body(i):
                # ---------- attention ----------
                qT = ps.tile([ATT, BPC], dt, name="qT", tag="psA")
                for hc in range(8):
                    nc.tensor.matmul(qT[:], s_lpT[:, hc * ATT:(hc + 1) * ATT],
                                     h2[:, 2 * hc:2 * hc + 2],
                                     start=(hc == 0), stop=(hc == 7))
                qTs = work.tile([ATT, BPC], dt, name="qTs")
                nc.vector.tensor_copy(qTs[:], qT[:])

                loc = ps.tile([ATT, BPC * T_ENC], dt, name="loc", tag="psB")
                nc.tensor.matmul(loc[:], s_w2T[:], im2col[:], start=True, stop=True)
                tmp = work.tile([ATT, BPC * T_ENC], dt, name="tmp")
                nc.vector.tensor_add(tmp[:], loc[:], s_peT[:])
                X = work.tile([ATT, BPC * T_ENC], BF16, name="X")
                for b in range(BPC):
                    nc.scalar.activation(X[:, b * T_ENC:(b + 1) * T_ENC],
                                         tmp[:, b * T_ENC:(b + 1) * T_ENC],
                                         Tanh, bias=qTs[:, b:b + 1])

                enf = ps.tile([1, BPC * T_ENC], dt, name="enf", tag="psC")
                nc.tensor.matmul(enf[:], s_ewT[:], X[:], start=True, stop=False)
                nc.tensor.matmul(enf[:], s_negc[:], s_unmask[:], start=False,
                                 stop=True)
                mexp = work.tile([1, BPC * T_ENC], dt, name="mexp")
                denom = work.tile([1, BPC], dt, name="denom")
                for b in range(BPC):
                    nc.scalar.activation(mexp[:, b * T_ENC:(b + 1) * T_ENC],
                                         enf[:, b * T_ENC:(b + 1) * T_ENC],
                                         Exp, accum_out=denom[:, b:b + 1])
                recip = work.tile([1, BPC], dt, name="recip")
                nc.vector.reciprocal(recip[:], denom[:])
                # broadcast recip to 128 partitions via rank-1 matmul
                recipP = ps.tile([128, BPC], dt, name="recipP", tag="psE")
                nc.tensor.matmul(recipP[:], s_ones[:], recip[:], start=True,
                                 stop=True)
                recipPs = work.tile([128, BPC], dt, name="recipPs")
                nc.vector.tensor_copy(recipPs[:], recipP[:])

                enT = ps.tile([128, 2 * BPC], dt, name="enT", tag="psA")
                for c in range(2 * BPC):
                    nc.tensor.matmul(enT[:, c:c + 1], X[:, c * 128:(c + 1) * 128],
                                     s_ewT[:], start=True, stop=True)
                mexpT = work.tile([128, 2 * BPC], BF16, name="mexpT")
                nc.scalar.activation(mexpT[:], enT[:], Exp)
                nc.vector.tensor_mul(mexpT[:], mexpT[:], s_maskT[:])

                ctxp = ps.tile([128, BPC * 4], dt, name="ctxp", tag="psD")
                for b in range(BPC):
                    for ec in range(4):
                        col = b * 4 + ec
                        for t2 in range(2):
                            off = ((b * 2 + t2) * 4 + ec) * 128
                            nc.tensor.matmul(
                                ctxp[:, col:col + 1], s_enc[:, off:off + 128],
                                mexpT[:, (b * 2 + t2):(b * 2 + t2) + 1],
                                start=(t2 == 0), stop=(t2 == 1))
                ctxT = work.tile([128, BPC * 4], BF16, name="ctxT")
                for b in range(BPC):
                    nc.vector.tensor_mul(
                        ctxT[:, b * 4:(b + 1) * 4], ctxp[:, b * 4:(b + 1) * 4],
                        recipPs[:, b:b + 1].broadcast_to([128, 4]))

                # ---------- LSTM layer 0 ----------
                g0 = ps.tile([128, 2 * NSLOT], dt, name="g0", tag="psB")
                xgv = bass.AP(tensor=x0gs.tensor,
                              offset=x0gs.offset + i * BPC,
                              ap=[x0gs.ap[0], [T_DEC * BPC, NSLOT], [1, BPC]])
                for s in range(NSLOT):
                    for ec in range(4):
                        woff = (ec * NSLOT + s) * 128
                        rhs = bass.AP(tensor=ctxT.tensor,
                                      offset=ctxT.offset + ec,
                                      ap=[ctxT.ap[0], [4, BPC]])
                        nc.tensor.matmul(g0[:, 2 * s:2 * s + 2],
                                         s_w0cT[:, woff:woff + 128], rhs,
                                         start=(ec == 0), stop=(ec == 3))
                T0 = work.tile([128, 2 * NSLOT], dt, name="T0")
                nc.vector.tensor_add(
                    T0[:].rearrange("p (s b) -> p s b", b=BPC), g0[:]
                    .rearrange("p (s b) -> p s b", b=BPC), xgv)
                nc.scalar.activation(T0[:], T0[:], Tanh)
                # c2 = (ti+1)*tg ; tc = tanh(0.5*c2) ; h1' = (to+1)*tc = 2*h1
                c2 = work.tile([128, 16], dt, name="c2")
                nc.vector.scalar_tensor_tensor(c2[:], T0[:, 0:16], 1.0,
                                               T0[:, 32:48], ADD, MUL)
                nc.scalar.activation(c2[:], c2[:], Tanh, scale=0.5)
                h1 = work.tile([128, 16], BF16, name="h1")
                nc.vector.scalar_tensor_tensor(h1[:], T0[:, 16:32], 1.0,
                                               c2[:], ADD, MUL)

                # ---------- LSTM layer 1 ----------
                g1 = ps.tile([128, 2 * NSLOT], dt, name="g1", tag="psC")
                for s in range(NSLOT):
                    for hc in range(8):
                        woff = (hc * NSLOT + s) * 128
                        nc.tensor.matmul(g1[:, 2 * s:2 * s + 2],
                                         s_w1T[:, woff:woff + 128],
                                         h1[:, 2 * hc:2 * hc + 2],
                                         start=(hc == 0), stop=(hc == 7))
                T1 = work.tile([128, 2 * NSLOT], dt, name="T1")
                nc.scalar.activation(T1[:], g1[:], Tanh)
                c3 = work.tile([128, 16], dt, name="c3")
                nc.vector.scalar_tensor_tensor(c3[:], T1[:, 0:16], 1.0,
                                               T1[:, 32:48], ADD, MUL)
                nc.scalar.activation(c3[:], c3[:], Tanh, scale=0.5)
                nc.vector.scalar_tensor_tensor(h2[:], T1[:, 16:32], 1.0,
                                               c3[:], ADD, MUL)

                # ---------- projection ----------
                melp = ps.tile([MEL + 1, BPC], dt, name="melp", tag="psD")
                for kc in range(12):
                    woff = kc * (MEL + 1)
                    if kc < 8:
                        rhs = h2[:, 2 * kc:2 * kc + 2]
                    else:
                        ec = kc - 8
                        rhs = bass.AP(tensor=ctxT.tensor,
                                      offset=ctxT.offset + ec,
                                      ap=[ctxT.ap[0], [4, BPC]])
                    nc.tensor.matmul(melp[:], s_decwT[:, woff:woff + MEL + 1],
                                     rhs, start=(kc == 0), stop=(kc == 11))
                nc.vector.tensor_copy(melbuf[:MEL + 1, bass.ts(i, BPC)], melp[:])

                # ---------- awc / im2col update for next step ----------
                aw = work.tile([1, BPC * T_ENC], dt, name="aw")
                for b in range(BPC):
                    nc.vector.tensor_mul(
                        aw[:, b * T_ENC:(b + 1) * T_ENC],
                        mexp[:, b * T_ENC:(b + 1) * T_ENC],
                        recip[:, b:b + 1].broadcast_to([1, T_ENC]))
                awc_v = bass.AP(tensor=awc.tensor, offset=awc.offset + PAD,
                                ap=[awc.ap[0], [TPADDED, BPC], [1, T_ENC]])
                nc.vector.tensor_add(
                    awc_v, awc_v, aw[:].rearrange("p (b t) -> p b t", b=BPC))
                for b in range(BPC):
                    src = bass.AP(tensor=awc.tensor,
                                  offset=awc.offset + b * TPADDED,
                                  ap=[awc.ap[0], [1, K], [1, T_ENC]])
                    dst = bass.AP(tensor=im2col.tensor,
                                  offset=im2col.offset + b * T_ENC,
                                  ap=[im2col.ap[0], [1, T_ENC]])
                    nc.sync.dma_start(out=dst, in_=src)

            with tc.For_i(0, T_DEC, 1,
                          hint_engines=(mybir.EngineType.PE,)) as i:
                step_body(i)

            nc.sync.dma_start(out=out_ext[:], in_=melbuf[:MEL + 1, :])

    nc.compile()
    return nc


def _prep_inputs(inputs):
    f32 = np.float32
    enc = np.asarray(inputs["encoder_output"], f32)
    mels = np.asarray(inputs["padded_mels"], f32)
    tl = np.asarray(inputs["text_lengths"]).astype(np.int64)

    enc_proj_w = np.asarray(inputs["enc_proj_w"], f32)
    lstm_proj_w = np.asarray(inputs["lstm_proj_w"], f32)
    loc_conv_w = np.asarray(inputs["loc_conv_w"], f32)
    loc_conv_b = np.asarray(inputs["loc_conv_b"], f32)
    loc_dense_w = np.asarray(inputs["loc_dense_w"], f32)
    loc_dense_b = np.asarray(inputs["loc_dense_b"], f32)
    e_w = np.asarray(inputs["e_w"], f32)
    e_b = np.asarray(inputs["e_b"], f32)
    prenet1_w = np.asarray(inputs["prenet1_w"], f32)
    prenet2_w = np.asarray(inputs["prenet2_w"], f32)
    w_ih0 = np.asarray(inputs["w_ih0"], f32)
    b_ih0 = np.asarray(inputs["b_ih0"], f32)
    b_hh0 = np.asarray(inputs["b_hh0"], f32)
    w_ih1 = np.asarray(inputs["w_ih1"], f32)
    b_ih1 = np.asarray(inputs["b_ih1"], f32)
    b_hh1 = np.asarray(inputs["b_hh1"], f32)
    proj_w = np.asarray(inputs["proj_w"], f32)
    proj_b = np.asarray(inputs["proj_b"], f32)
    stop_w = np.asarray(inputs["stop_w"], f32)
    stop_b = np.asarray(inputs["stop_b"], f32)

    # keep (i, o, g) rows, drop dead f; scale i,o rows by 0.5 (sigmoid trick)
    def iog(w):
        wi, wf, wg, wo = np.split(w, 4, axis=0)
        return np.concatenate([wi * 0.5, wo * 0.5, wg], axis=0)

    w0 = iog(w_ih0)
    b0 = iog((b_ih0 + b_hh0)[:, None])[:, 0]
    w1 = iog(w_ih1) * 0.5          # extra 0.5: consumes h1' = 2*h1
    b1 = iog((b_ih1 + b_hh1)[:, None])[:, 0]
    assert np.abs(b1).max() == 0.0, "nonzero LSTM1 bias not supported"
    w0x, w0c = w0[:, :PRE], w0[:, PRE:]

    prev = np.concatenate([np.zeros((B, 1, MEL), f32), mels[:, :-1]], axis=1)
    x = np.maximum(prev @ prenet1_w.T, 0.0)
    x = np.maximum(x @ prenet2_w.T, 0.0)         # [16, 200, 256]
    assert np.abs(b0).max() == 0.0, "nonzero LSTM0 bias not supported"

    W2 = loc_dense_w @ loc_conv_w[:, 0, :]       # [128, 31]
    pe_bias = loc_dense_b + loc_dense_w @ loc_conv_b
    pe = enc @ enc_proj_w.T + pe_bias            # [16, 256, 128]
    mask = (np.arange(T_ENC)[None, :] < tl[:, None])

    # en = X . e_w + e_b : fold e_b into the mask matmul? simpler: e_b is a
    # constant added to every energy -> softmax invariant; drop it.
    del e_b

    decw_h = 0.5 * np.concatenate([proj_w[:, :H], stop_w[:, :H]], axis=0)
    decw_c = np.concatenate([proj_w[:, H:], stop_w[:, H:]], axis=0)
    decb = np.concatenate([proj_b, stop_b], axis=0)
    assert np.abs(decb).max() == 0.0, "nonzero proj bias not supported"
    decw = np.concatenate([decw_h, decw_c], axis=1)   # [81, 1536]

    lpTm = (0.5 * lstm_proj_w).T                 # [1024, 128]

    base = {
        "lpT": np.ascontiguousarray(
            lpTm.reshape(8, 128, ATT).transpose(1, 0, 2).reshape(128, 8 * ATT)
            .astype(NPBF)),
        "w2T": np.ascontiguousarray(W2.T),
        "ewT": np.ascontiguousarray(e_w.T.astype(NPBF)),
        "negc": np.full((1, 1), NEG, NPBF),
        "onesrow": np.ones((1, 128), f32),
        "w0cT": np.ascontiguousarray(
            w0c.T.reshape(4, 128, NSLOT, 128).transpose(1, 0, 2, 3)
            .reshape(128, 4 * NSLOT * 128).astype(NPBF)),
        "w1T": np.ascontiguousarray(
            w1.T.reshape(8, 128, NSLOT, 128).transpose(1, 0, 2, 3)
            .reshape(128, 8 * NSLOT * 128).astype(NPBF)),
        "decwT": np.ascontiguousarray(
            decw.T.reshape(12, 128, MEL + 1).transpose(1, 0, 2)
            .reshape(128, 12 * (MEL + 1)).astype(NPBF)),
        "w0xT": np.ascontiguousarray(
            w0x.T.reshape(2, 128, NSLOT, 128).transpose(1, 0, 2, 3)
            .reshape(128, 2 * NSLOT * 128).astype(NPBF)),
    }
    in_maps = []
    for c in range(NCORES):
        bs = slice(c * BPC, (c + 1) * BPC)
        m = dict(base)
        mk = mask[bs]
        m["unmask"] = np.ascontiguousarray(
            (~mk).astype(NPBF).reshape(1, BPC * T_ENC))
        m["maskT"] = np.ascontiguousarray(
            mk.reshape(BPC, 2, 128).transpose(2, 0, 1).reshape(128, BPC * 2)
            .astype(NPBF))
        m["peT"] = np.ascontiguousarray(
            pe[bs].reshape(BPC * T_ENC, ATT).T.astype(NPBF))
        m["encT"] = np.ascontiguousarray(
            enc[bs].reshape(BPC, 2, 128, 4, 128).transpose(2, 0, 1, 3, 4)
            .reshape(128, BPC * 2 * 4 * 128).astype(NPBF))
        # xT[p, (pc, t, b)] = x[b, t, pc*128+p]
        m["xT"] = np.ascontiguousarray(
            x[bs].transpose(2, 1, 0).reshape(2, 128, T_DEC, BPC)
            .transpose(1, 0, 2, 3).reshape(128, 2 * T_DEC * BPC).astype(NPBF))
        in_maps.append(m)
    return in_maps


def kernel(**inputs):
    if "nc" not in _cache:
        _cache["nc"] = _build_program()
    nc = _cache["nc"]
    in_maps = _prep_inputs(inputs)
    res = run_bass_kernel_spmd(nc, in_maps, core_ids=list(range(NCORES)))
    mels = np.zeros((B, T_DEC, MEL), np.float32)
    stops = np.zeros((B, T_DEC), np.float32)
    for c in range(NCORES):
        o = res.results[c]["out"].reshape(MEL + 1, T_DEC, BPC)
        for b in range(BPC):
            mels[c * BPC + b] = o[:MEL, :, b].T
            stops[c * BPC + b] = o[MEL, :, b]
    return mels, stops
